# revision 1
# baseline (speedup 1.0000x reference)
"""HashSoftmax (embedding_lookup) Trainium2 Bass kernel.

Strategy (vocab-sharded tensor parallel over 8 NeuronCores):
  - Each core owns a 4000-entry vocab shard (padded to 4096 = 32 tiles of 128).
  - pool is replicated (bf16), x is replicated (pre-transposed bf16 [256, 4096]).
  - Per 128-vocab tile: 20 indirect DMA gathers fetch pool rows for each hash
    slot into SBUF [128v, 20j*256h] (bf16); a fused DVE
    scalar_tensor_tensor chain does emb[v] = sum_j w[v,j]*G[v,j,:] in f32;
    PE transposes emb to embed_T [h, v] (bf16); the main bf16 matmul
    x_T.T @ embed_T accumulates logits in PSUM over 2 h-chunks; ACT copies
    PSUM->SBUF; HWDGE DMA writes the [4096, 4096] f32 logit shard.
  - Host concatenates the 8 shards -> [2, 2048, 32000] f32.
"""

import numpy as np
import ml_dtypes

import concourse.bass as bass
import concourse.mybir as mybir
import concourse.tile as tile
import concourse.bacc as bacc
from concourse.bass_utils import run_bass_kernel_spmd
from concourse.masks import make_identity

F32 = mybir.dt.float32
BF16 = mybir.dt.bfloat16
I32 = mybir.dt.int32

VOCAB, HIDDEN, POOL, NHASH = 32000, 256, 100000, 20
N_CORES = 8
T = 4096                 # tokens = 2*2048
VC = 4096                # padded vocab per core (real 4000)
TILES = VC // 128        # 32 vocab tiles per core
VB_TILES = 4             # vocab tiles per matmul block (512 cols)
N_VB = TILES // VB_TILES # 8 blocks
J = NHASH
H = HIDDEN

_NC_CACHE = {}


def _build_nc():
    nc = bacc.Bacc("TRN2", target_bir_lowering=False, debug=False)

    pool_d = nc.dram_tensor("pool", [POOL, H], BF16, kind="ExternalInput")
    xT_d = nc.dram_tensor("xT", [H, T], BF16, kind="ExternalInput")
    hidx_d = nc.dram_tensor("hidx", [128, TILES * J], I32, kind="ExternalInput")
    widx_d = nc.dram_tensor("widx", [128, TILES * J], F32, kind="ExternalInput")
    out_d = nc.dram_tensor("out", [T, VC], F32, kind="ExternalOutput")

    with tile.TileContext(nc) as tc:
        with (
            tc.tile_pool(name="const", bufs=1) as const_pool,
            tc.tile_pool(name="gather", bufs=3) as g_pool,
            tc.tile_pool(name="emb", bufs=3) as emb_pool,
            tc.tile_pool(name="embT", bufs=2) as embT_pool,
            tc.tile_pool(name="osb", bufs=4) as out_pool,
            tc.tile_pool(name="psum_tr", bufs=2, space="PSUM") as psum_tr,
            tc.tile_pool(name="psum_mm", bufs=3, space="PSUM") as psum_mm,
        ):
            ident = const_pool.tile([128, 128], F32)
            make_identity(nc, ident[:])

            xT_sb = const_pool.tile([128, 2, T], BF16)
            for hc in range(2):
                nc.sync.dma_start(
                    out=xT_sb[:, hc, :], in_=xT_d[hc * 128:(hc + 1) * 128, :]
                )
            hidx_sb = const_pool.tile([128, TILES * J], I32)
            nc.sync.dma_start(out=hidx_sb[:], in_=hidx_d[:])
            widx_sb = const_pool.tile([128, TILES * J], F32)
            nc.sync.dma_start(out=widx_sb[:], in_=widx_d[:])

            for vb in range(N_VB):
                embT = embT_pool.tile([128, 2, VB_TILES * 128], BF16)
                for s in range(VB_TILES):
                    ti = vb * VB_TILES + s
                    G = g_pool.tile([128, J * H], BF16)
                    for j in range(J):
                        # one descriptor per partition: gathers pool[idx[p], :]
                        # into G[p, j*H:(j+1)*H]  (HW-validated pattern)
                        nc.gpsimd.indirect_dma_start(
                            out=G[:, j * H:(j + 1) * H],
                            out_offset=None,
                            in_=pool_d[:],
                            in_offset=bass.IndirectOffsetOnAxis(
                                ap=hidx_sb[:, ti * J + j:ti * J + j + 1], axis=0
                            ),
                        )
                    emb = emb_pool.tile([128, H], F32)
                    nc.vector.tensor_scalar_mul(
                        emb[:], G[:, 0:H], widx_sb[:, ti * J:ti * J + 1]
                    )
                    for j in range(1, J):
                        nc.vector.scalar_tensor_tensor(
                            out=emb[:],
                            in0=G[:, j * H:(j + 1) * H],
                            scalar=widx_sb[:, ti * J + j:ti * J + j + 1],
                            in1=emb[:],
                            op0=mybir.AluOpType.mult,
                            op1=mybir.AluOpType.add,
                        )
                    for hc in range(2):
                        ptr = psum_tr.tile([128, 128], F32)
                        nc.tensor.transpose(
                            out=ptr[:],
                            in_=emb[:, hc * 128:(hc + 1) * 128],
                            identity=ident[:],
                        )
                        nc.vector.tensor_copy(
                            out=embT[:, hc, s * 128:(s + 1) * 128], in_=ptr[:]
                        )

                for t in range(TILES):
                    pmm = psum_mm.tile([128, 512], F32)
                    for hc in range(2):
                        nc.tensor.matmul(
                            out=pmm[:],
                            lhsT=xT_sb[:, hc, t * 128:(t + 1) * 128],
                            rhs=embT[:, hc, :],
                            start=(hc == 0),
                            stop=(hc == 1),
                        )
                    osb = out_pool.tile([128, 512], F32)
                    nc.scalar.copy(osb[:], pmm[:])
                    nc.sync.dma_start(
                        out=out_d[t * 128:(t + 1) * 128, vb * 512:(vb + 1) * 512],
                        in_=osb[:],
                    )
    nc.compile()
    return nc


def _get_nc():
    if "nc" not in _NC_CACHE:
        _NC_CACHE["nc"] = _build_nc()
    return _NC_CACHE["nc"]


def kernel(x, pool, import_params, hash_values, _trace=False):
    x = np.asarray(x)
    pool = np.asarray(pool)
    import_params = np.asarray(import_params, dtype=np.float32)
    hash_values = np.asarray(hash_values)

    xT_bf = np.ascontiguousarray(
        x.reshape(T, H).astype(np.float32).T
    ).astype(ml_dtypes.bfloat16)
    pool_bf = pool.astype(ml_dtypes.bfloat16)

    vc_real = VOCAB // N_CORES  # 4000
    in_maps = []
    for c in range(N_CORES):
        hv = hash_values[c * vc_real:(c + 1) * vc_real].astype(np.int32)
        wv = import_params[c * vc_real:(c + 1) * vc_real]
        hv_p = np.zeros((VC, J), np.int32)
        wv_p = np.zeros((VC, J), np.float32)
        hv_p[:vc_real] = hv
        wv_p[:vc_real] = wv
        # [VC, J] -> [128, TILES*J] partition-major: [p, ti*J+j] = row ti*128+p
        hidx = np.ascontiguousarray(
            hv_p.reshape(TILES, 128, J).transpose(1, 0, 2).reshape(128, TILES * J)
        )
        widx = np.ascontiguousarray(
            wv_p.reshape(TILES, 128, J).transpose(1, 0, 2).reshape(128, TILES * J)
        )
        in_maps.append(
            {"pool": pool_bf, "xT": xT_bf, "hidx": hidx, "widx": widx}
        )

    nc = _get_nc()
    res = run_bass_kernel_spmd(
        nc, in_maps, list(range(N_CORES)), trace=_trace
    )
    out = np.empty((T, VOCAB), np.float32)
    for c in range(N_CORES):
        out[:, c * vc_real:(c + 1) * vc_real] = res.results[c]["out"][:, :vc_real]
    result = out.reshape(2, 2048, VOCAB)
    if _trace:
        return result, res
    return result


# revision 2
# speedup vs baseline: 1.1729x; 1.1729x over previous
"""HashSoftmax (embedding_lookup) Trainium2 Bass kernel.

Strategy (vocab-sharded tensor parallel over 8 NeuronCores):
  - Each core owns a 4000-entry vocab shard (padded to 4096 = 32 tiles of 128).
  - pool is replicated (bf16), x is replicated (pre-transposed bf16 [256, 4096]).
  - Per 128-vocab tile: 20 indirect DMA gathers fetch pool rows for each hash
    slot into SBUF [128v, 20j*256h] (bf16); a fused DVE
    scalar_tensor_tensor chain does emb[v] = sum_j w[v,j]*G[v,j,:] in f32;
    PE transposes emb to embed_T [h, v] (bf16); the main bf16 matmul
    x_T.T @ embed_T accumulates logits in PSUM over 2 h-chunks; ACT copies
    PSUM->SBUF; HWDGE DMA writes the [4096, 4096] f32 logit shard.
  - Host concatenates the 8 shards -> [2, 2048, 32000] f32.
"""

import os

import numpy as np
import ml_dtypes

# No NTFF/axon profiling hook exists in this container (antenv.axon_hooks is
# absent); a stray BASS_TRACE env would crash run_bass_kernel_spmd otherwise.
os.environ.setdefault("BASS_NEVER_TRACE", "1")

import concourse.bass as bass
import concourse.mybir as mybir
import concourse.tile as tile
import concourse.bacc as bacc
from concourse.bass_utils import run_bass_kernel_spmd
from concourse.masks import make_identity

F32 = mybir.dt.float32
BF16 = mybir.dt.bfloat16
I32 = mybir.dt.int32

VOCAB, HIDDEN, POOL, NHASH = 32000, 256, 100000, 20
N_CORES = 8
T = 4096                 # tokens = 2*2048
VC = 4096                # padded vocab per core (real 4000)
TILES = VC // 128        # 32 vocab tiles per core
VB_TILES = 4             # vocab tiles per matmul block (512 cols)
N_VB = TILES // VB_TILES # 8 blocks
J = NHASH
H = HIDDEN

_NC_CACHE = {}


def _build_nc():
    nc = bacc.Bacc("TRN2", target_bir_lowering=False, debug=False)

    pool_d = nc.dram_tensor("pool", [POOL, H], BF16, kind="ExternalInput")
    xT_d = nc.dram_tensor("xT", [H, T], BF16, kind="ExternalInput")
    hidx_d = nc.dram_tensor("hidx", [128, TILES * J], I32, kind="ExternalInput")
    widx_d = nc.dram_tensor("widx", [128, TILES * J], F32, kind="ExternalInput")
    out_d = nc.dram_tensor("out", [T, VC], F32, kind="ExternalOutput")

    with tile.TileContext(nc) as tc:
        with (
            tc.tile_pool(name="const", bufs=1) as const_pool,
            tc.tile_pool(name="gather", bufs=3) as g_pool,
            tc.tile_pool(name="emb", bufs=3) as emb_pool,
            tc.tile_pool(name="embT", bufs=2) as embT_pool,
            tc.tile_pool(name="osb", bufs=4) as out_pool,
            tc.tile_pool(name="psum_tr", bufs=2, space="PSUM") as psum_tr,
            tc.tile_pool(name="psum_mm", bufs=3, space="PSUM") as psum_mm,
        ):
            ident = const_pool.tile([128, 128], F32)
            make_identity(nc, ident[:])

            xT_sb = const_pool.tile([128, 2, T], BF16)
            for hc in range(2):
                nc.sync.dma_start(
                    out=xT_sb[:, hc, :], in_=xT_d[hc * 128:(hc + 1) * 128, :]
                )
            hidx_sb = const_pool.tile([128, TILES * J], I32)
            nc.sync.dma_start(out=hidx_sb[:], in_=hidx_d[:])
            widx_sb = const_pool.tile([128, TILES * J], F32)
            nc.sync.dma_start(out=widx_sb[:], in_=widx_d[:])

            for vb in range(N_VB):
                embT = embT_pool.tile([128, 2, VB_TILES * 128], BF16)
                for s in range(VB_TILES):
                    ti = vb * VB_TILES + s
                    G = g_pool.tile([128, J * H], BF16)
                    for j in range(J):
                        # one descriptor per partition: gathers pool[idx[p], :]
                        # into G[p, j*H:(j+1)*H]  (HW-validated pattern)
                        nc.gpsimd.indirect_dma_start(
                            out=G[:, j * H:(j + 1) * H],
                            out_offset=None,
                            in_=pool_d[:],
                            in_offset=bass.IndirectOffsetOnAxis(
                                ap=hidx_sb[:, ti * J + j:ti * J + j + 1], axis=0
                            ),
                        )
                    emb = emb_pool.tile([128, H], F32)
                    nc.vector.tensor_scalar_mul(
                        emb[:], G[:, 0:H], widx_sb[:, ti * J:ti * J + 1]
                    )
                    for j in range(1, J):
                        nc.vector.scalar_tensor_tensor(
                            out=emb[:],
                            in0=G[:, j * H:(j + 1) * H],
                            scalar=widx_sb[:, ti * J + j:ti * J + j + 1],
                            in1=emb[:],
                            op0=mybir.AluOpType.mult,
                            op1=mybir.AluOpType.add,
                        )
                    for hc in range(2):
                        ptr = psum_tr.tile([128, 128], F32)
                        nc.tensor.transpose(
                            out=ptr[:],
                            in_=emb[:, hc * 128:(hc + 1) * 128],
                            identity=ident[:],
                        )
                        nc.vector.tensor_copy(
                            out=embT[:, hc, s * 128:(s + 1) * 128], in_=ptr[:]
                        )

                for t in range(TILES):
                    pmm = psum_mm.tile([128, 512], F32)
                    for hc in range(2):
                        nc.tensor.matmul(
                            out=pmm[:],
                            lhsT=xT_sb[:, hc, t * 128:(t + 1) * 128],
                            rhs=embT[:, hc, :],
                            start=(hc == 0),
                            stop=(hc == 1),
                        )
                    osb = out_pool.tile([128, 512], F32)
                    nc.scalar.copy(osb[:], pmm[:])
                    nc.sync.dma_start(
                        out=out_d[t * 128:(t + 1) * 128, vb * 512:(vb + 1) * 512],
                        in_=osb[:],
                    )
    nc.compile()
    return nc


def _get_nc():
    if "nc" not in _NC_CACHE:
        _NC_CACHE["nc"] = _build_nc()
    return _NC_CACHE["nc"]


def kernel(x, pool, import_params, hash_values, _trace=False):
    x = np.asarray(x)
    pool = np.asarray(pool)
    import_params = np.asarray(import_params, dtype=np.float32)
    hash_values = np.asarray(hash_values)

    xT_bf = np.ascontiguousarray(
        x.reshape(T, H).astype(np.float32).T
    ).astype(ml_dtypes.bfloat16)
    pool_bf = pool.astype(ml_dtypes.bfloat16)

    vc_real = VOCAB // N_CORES  # 4000
    in_maps = []
    for c in range(N_CORES):
        hv = hash_values[c * vc_real:(c + 1) * vc_real].astype(np.int32)
        wv = import_params[c * vc_real:(c + 1) * vc_real]
        hv_p = np.zeros((VC, J), np.int32)
        wv_p = np.zeros((VC, J), np.float32)
        hv_p[:vc_real] = hv
        wv_p[:vc_real] = wv
        # [VC, J] -> [128, TILES*J] partition-major: [p, ti*J+j] = row ti*128+p
        hidx = np.ascontiguousarray(
            hv_p.reshape(TILES, 128, J).transpose(1, 0, 2).reshape(128, TILES * J)
        )
        widx = np.ascontiguousarray(
            wv_p.reshape(TILES, 128, J).transpose(1, 0, 2).reshape(128, TILES * J)
        )
        in_maps.append(
            {"pool": pool_bf, "xT": xT_bf, "hidx": hidx, "widx": widx}
        )

    nc = _get_nc()
    res = run_bass_kernel_spmd(
        nc, in_maps, list(range(N_CORES)), trace=_trace
    )
    out = np.empty((T, VOCAB), np.float32)
    for c in range(N_CORES):
        out[:, c * vc_real:(c + 1) * vc_real] = res.results[c]["out"][:, :vc_real]
    result = out.reshape(2, 2048, VOCAB)
    if _trace:
        return result, res
    return result
